# revision 5
# baseline (speedup 1.0000x reference)
"""Trainium2 Bass kernel for nn_CMIP_75883482186148 (histogram_binning).

Reference semantics: thresholds t1/t2 are found by a histogram-valley search
over |w1|/|w2| (C=256 channels); channel masks m1 = |w1|>=t1, m2 = |w2|>=t2;
then over [B=8, C=256, H=128, W=128] f32 tensors:
    y1 = where(m1[None,:,None,None], x0, x1)
    y2 = where(m2[None,:,None,None], x1, x0)

Every output channel is a verbatim copy of one input's channel slab, so the
device work is pure data movement.  Strategy:

  * The O(C) threshold search is bit-exactly ported to host float32 numpy and
    computed as kernel launch parameters (it decides the DMA pattern).
  * Batch is sharded across the 8 NeuronCores (1 batch element each, SPMD).
  * In-place outputs: inputs are donated to the jit, and jax pairs each
    donated input with the equal-shaped output (y1 <- x0's device buffer,
    y2 <- x1's), which libneuronpjrt honors for the wrapped bass NEFF.  The
    NEFF then only patches the channels where the output differs from the
    aliased input: y1 takes x1 on ~m1 channels, y2 takes x0 on ~m2 channels.
    Channels in S = ~m1 & ~m2 swap between the two buffers and stage through
    an internal DRAM scratch first.  Moved bytes per core:
    (|~m1| + |~m2| + 2|S|) * 64 KiB, typically ~1-4% of the 64 MiB a full
    rewrite would move (the reference's masks are heavily skewed).
  * Patch DMAs are DRAM->DRAM on the two HWDGE rings (SP + ACT) so issue
    serialization is halved; equal-length patch runs are pairwise merged
    into single strided-AP DMAs to cut instruction count further.
"""

import numpy as np

B, C, H, W = 8, 256, 128, 128
F = H * W  # contiguous f32 elements per (batch, channel) slab
N_CORES = 8

_FN_CACHE: dict = {}


def _mask(w: np.ndarray) -> np.ndarray:
    """Bit-exact float32 port of reference.search_threshold + (|w| >= t)."""
    b = np.abs(np.asarray(w, dtype=np.float32))
    bins = b.shape[0]
    wmin = b.min()
    wmax = b.max()
    idx = np.clip(
        np.floor((b - wmin) / (wmax - wmin) * np.float32(bins)).astype(np.int32),
        0,
        bins - 1,
    )
    hist = np.zeros(bins, dtype=np.float32)
    np.add.at(hist, idx, np.float32(1))
    d = np.diff(hist)
    cond = (d[:-1] <= 0) & (d[1:] > 0)
    i = np.int32(np.argmax(cond)) if cond.any() else np.int32(0)
    t = wmin + np.float32(i + 2) * (wmax - wmin) / np.float32(bins)
    return b >= t


def _runs(mask: np.ndarray, value: bool | None = None):
    """Maximal runs of equal mask value: [(start, end, value)].
    If `value` given, only runs with that value, as [(start, end)]."""
    out = []
    s = 0
    n = len(mask)
    for c in range(1, n + 1):
        if c == n or bool(mask[c]) != bool(mask[s]):
            out.append((s, c, bool(mask[s])))
            s = c
    if value is None:
        return out
    return [(a, b) for a, b, v in out if v == value]


def _merge_runs(runs):
    """Group equal-length runs into strided groups: [(starts, length)] where
    starts form an arithmetic progression (any two equal-length runs do)."""
    by_len: dict = {}
    for a, b in runs:
        by_len.setdefault(b - a, []).append(a)
    groups = []
    for k, starts in by_len.items():
        starts.sort()
        i = 0
        while i < len(starts):
            # greedily extend an arithmetic progression
            if i + 1 < len(starts):
                step = starts[i + 1] - starts[i]
                j = i + 1
                while j + 1 < len(starts) and starts[j + 1] - starts[j] == step:
                    j += 1
                groups.append((starts[i], j - i + 1, step, k))
                i = j + 1
            else:
                groups.append((starts[i], 1, 0, k))
                i += 1
    return groups  # (first_start, count, step, length)


def _build_patch_program(m1: np.ndarray, m2: np.ndarray):
    """Patch-only program: y1/y2 are bound to x0/x1's buffers by donation
    aliasing; only differing channels are written.  S-channels (both masks
    False) swap data between the buffers, so they stage via DRAM scratch."""
    import concourse.bass as bass
    import concourse.mybir as mybir

    f32 = mybir.dt.float32
    nc = bass.Bass(trn_type="TRN2", enable_partition_id=False)
    x0 = nc.dram_tensor("x0", [C, F], f32, kind="ExternalInput")
    x1 = nc.dram_tensor("x1", [C, F], f32, kind="ExternalInput")
    y1 = nc.dram_tensor("y1", [C, F], f32, kind="ExternalOutput")
    y2 = nc.dram_tensor("y2", [C, F], f32, kind="ExternalOutput")

    s_mask = (~m1) & (~m2)  # swap channels: y1[c]<-x1[c] AND y2[c]<-x0[c]
    s_runs = _runs(s_mask, True)
    s_total = int(s_mask.sum())
    # direct patches: source channel is never overwritten by the other side
    p1_groups = _merge_runs(_runs((~m1) & m2, True))  # y1 <- x1
    p2_groups = _merge_runs(_runs((~m2) & m1, True))  # y2 <- x0

    def ap(t, start, count, step, length):
        # channels [start + i*step : +length) for i in range(count), flat view
        if count == 1:
            return t[start : start + length, :]
        return bass.AP(
            t, start * F, [[step * F, count], [1, length * F]]
        )

    direct = [(y1, x1, g) for g in p1_groups] + [(y2, x0, g) for g in p2_groups]
    # round-robin across the two HWDGE rings, largest first for balance
    direct.sort(key=lambda d: -(d[2][1] * d[2][3]))
    direct_sp = direct[0::2]
    direct_act = direct[1::2]

    scr0 = scr1 = None
    if s_total:
        scr0 = nc.dram_tensor("scr0", [s_total, F], f32, kind="Internal")
        scr1 = nc.dram_tensor("scr1", [s_total, F], f32, kind="Internal")

    with (
        nc.semaphore("dma1") as s1,
        nc.semaphore("dma2") as s2,
        nc.Block() as block,
    ):

        @block.sync
        def _(sync):
            n = 0
            # stage the swap set first (reads of both buffers)
            o = 0
            for a, b in s_runs:
                k = b - a
                sync.dma_start(scr0[o : o + k, :], x0[a:b, :]).then_inc(s1, 16)
                sync.dma_start(scr1[o : o + k, :], x1[a:b, :]).then_inc(s1, 16)
                n += 32
                o += k
            n_stage = n
            # direct patches can go while staging drains
            for dst, src, (a, cnt, st, k) in direct_sp:
                sync.dma_start(
                    ap(dst, a, cnt, st, k), ap(src, a, cnt, st, k)
                ).then_inc(s1, 16)
                n += 16
            if s_total:
                # swap-set writes must wait for the staged reads
                sync.wait_ge(s1, n_stage)
                o = 0
                for a, b in s_runs:
                    k = b - a
                    sync.dma_start(y1[a:b, :], scr1[o : o + k, :]).then_inc(s1, 16)
                    sync.dma_start(y2[a:b, :], scr0[o : o + k, :]).then_inc(s1, 16)
                    n += 32
                    o += k
            if n:
                sync.wait_ge(s1, n)

        @block.scalar
        def _(scalar):
            n = 0
            for dst, src, (a, cnt, st, k) in direct_act:
                scalar.dma_start(
                    ap(dst, a, cnt, st, k), ap(src, a, cnt, st, k)
                ).then_inc(s2, 16)
                n += 16
            if n:
                scalar.wait_ge(s2, n)

    return nc


def _get_fn(key, m1, m2):
    cached = _FN_CACHE.get(key)
    if cached is not None:
        return cached

    import jax
    from jax.experimental.shard_map import shard_map
    from jax.sharding import Mesh, PartitionSpec as P

    from concourse.bass2jax import _bass_exec_p, install_neuronx_cc_hook

    install_neuronx_cc_hook()
    nc = _build_patch_program(m1, m2)
    aval = jax.core.ShapedArray((C, F), np.float32)

    def _body(a0, a1):
        outs = _bass_exec_p.bind(
            a0,
            a1,
            out_avals=(aval, aval),
            in_names=("x0", "x1"),
            out_names=("y1", "y2"),
            lowering_input_output_aliases=(),
            sim_require_finite=True,
            sim_require_nnan=True,
            nc=nc,
        )
        return tuple(outs)

    devices = jax.devices()[:N_CORES]
    assert len(devices) == N_CORES, f"need {N_CORES} cores, got {len(devices)}"
    mesh = Mesh(np.asarray(devices), ("core",))
    # donating x0/x1 makes jax alias them to the equal-shaped outputs
    # (y1<-x0, y2<-x1, first-fit in declaration order) — verified bit-exact.
    fn = jax.jit(
        shard_map(
            _body,
            mesh=mesh,
            in_specs=(P("core"), P("core")),
            out_specs=(P("core"), P("core")),
            check_rep=False,
        ),
        donate_argnums=(0, 1),
    )
    _FN_CACHE[key] = fn
    return fn


def kernel(x0, x1, w1, w2):
    x0 = np.ascontiguousarray(np.asarray(x0, dtype=np.float32))
    x1 = np.ascontiguousarray(np.asarray(x1, dtype=np.float32))
    assert x0.shape == (B, C, H, W) and x1.shape == (B, C, H, W)

    m1 = _mask(w1)
    m2 = _mask(w2)
    key = (m1.tobytes(), m2.tobytes())
    fn = _get_fn(key, m1, m2)
    o1, o2 = fn(x0.reshape(B * C, F), x1.reshape(B * C, F))
    y1 = np.asarray(o1).reshape(B, C, H, W)
    y2 = np.asarray(o2).reshape(B, C, H, W)
    return (y1, y2)


# revision 7
# speedup vs baseline: 1.9390x; 1.9390x over previous
"""Trainium2 Bass kernel for nn_CMIP_75883482186148 (histogram_binning).

Reference semantics: thresholds t1/t2 are found by a histogram-valley search
over |w1|/|w2| (C=256 channels); channel masks m1 = |w1|>=t1, m2 = |w2|>=t2;
then over [B=8, C=256, H=128, W=128] f32 tensors:
    y1 = where(m1[None,:,None,None], x0, x1)
    y2 = where(m2[None,:,None,None], x1, x0)

Every output channel is a verbatim copy of one input's channel slab, so the
device work is pure data movement.  Strategy:

  * The O(C) threshold search is bit-exactly ported to host float32 numpy and
    computed as kernel launch parameters (it decides the DMA pattern).
  * Batch is sharded across the 8 NeuronCores (1 batch element each, SPMD).
  * In-place outputs: inputs are donated to the jit, and jax pairs each
    donated input with the equal-shaped output (y1 <- x0's device buffer,
    y2 <- x1's), which libneuronpjrt honors for the wrapped bass NEFF.  The
    NEFF then only patches the channels where the output differs from the
    aliased input: y1 takes x1 on ~m1 channels, y2 takes x0 on ~m2 channels.
    Channels in S = ~m1 & ~m2 swap between the two buffers and stage through
    an internal DRAM scratch first.  Moved bytes per core:
    (|~m1| + |~m2| + 2|S|) * 64 KiB, typically ~1-4% of the 64 MiB a full
    rewrite would move (the reference's masks are heavily skewed).
  * Patch DMAs are DRAM->DRAM on the two HWDGE rings (SP + ACT) so issue
    serialization is halved; equal-length patch runs are pairwise merged
    into single strided-AP DMAs to cut instruction count further.
"""

import numpy as np

B, C, H, W = 8, 256, 128, 128
F = H * W  # contiguous f32 elements per (batch, channel) slab
N_CORES = 8

_FN_CACHE: dict = {}


def _mask(w: np.ndarray) -> np.ndarray:
    """Bit-exact float32 port of reference.search_threshold + (|w| >= t)."""
    b = np.abs(np.asarray(w, dtype=np.float32))
    bins = b.shape[0]
    wmin = b.min()
    wmax = b.max()
    idx = np.clip(
        np.floor((b - wmin) / (wmax - wmin) * np.float32(bins)).astype(np.int32),
        0,
        bins - 1,
    )
    hist = np.zeros(bins, dtype=np.float32)
    np.add.at(hist, idx, np.float32(1))
    d = np.diff(hist)
    cond = (d[:-1] <= 0) & (d[1:] > 0)
    i = np.int32(np.argmax(cond)) if cond.any() else np.int32(0)
    t = wmin + np.float32(i + 2) * (wmax - wmin) / np.float32(bins)
    return b >= t


def _runs(mask: np.ndarray, value: bool | None = None):
    """Maximal runs of equal mask value: [(start, end, value)].
    If `value` given, only runs with that value, as [(start, end)]."""
    out = []
    s = 0
    n = len(mask)
    for c in range(1, n + 1):
        if c == n or bool(mask[c]) != bool(mask[s]):
            out.append((s, c, bool(mask[s])))
            s = c
    if value is None:
        return out
    return [(a, b) for a, b, v in out if v == value]


def _merge_runs(runs):
    """Group equal-length runs into strided groups: [(starts, length)] where
    starts form an arithmetic progression (any two equal-length runs do)."""
    by_len: dict = {}
    for a, b in runs:
        by_len.setdefault(b - a, []).append(a)
    groups = []
    for k, starts in by_len.items():
        starts.sort()
        i = 0
        while i < len(starts):
            # greedily extend an arithmetic progression
            if i + 1 < len(starts):
                step = starts[i + 1] - starts[i]
                j = i + 1
                while j + 1 < len(starts) and starts[j + 1] - starts[j] == step:
                    j += 1
                groups.append((starts[i], j - i + 1, step, k))
                i = j + 1
            else:
                groups.append((starts[i], 1, 0, k))
                i += 1
    return groups  # (first_start, count, step, length)


def _build_patch_program(m1: np.ndarray, m2: np.ndarray):
    """Patch-only program: y1/y2 are bound to x0/x1's buffers by donation
    aliasing; only differing channels are written.  S-channels (both masks
    False) swap data between the buffers, so they stage via DRAM scratch."""
    import concourse.bass as bass
    import concourse.mybir as mybir

    f32 = mybir.dt.float32
    nc = bass.Bass(trn_type="TRN2", enable_partition_id=False)
    x0 = nc.dram_tensor("x0", [C, F], f32, kind="ExternalInput")
    x1 = nc.dram_tensor("x1", [C, F], f32, kind="ExternalInput")
    y1 = nc.dram_tensor("y1", [C, F], f32, kind="ExternalOutput")
    y2 = nc.dram_tensor("y2", [C, F], f32, kind="ExternalOutput")

    s_mask = (~m1) & (~m2)  # swap channels: y1[c]<-x1[c] AND y2[c]<-x0[c]
    s_runs = _runs(s_mask, True)
    s_total = int(s_mask.sum())
    # direct patches: source channel is never overwritten by the other side.
    # NOTE: keep each run as its own contiguous DMA — merging equal-length
    # runs into strided 2-count APs measured ~10us SLOWER completion (the
    # non-contiguous outer dim defeats the 16-engine contiguous split).
    p1_groups = [(a, 1, 0, b - a) for a, b in _runs((~m1) & m2, True)]  # y1 <- x1
    p2_groups = [(a, 1, 0, b - a) for a, b in _runs((~m2) & m1, True)]  # y2 <- x0

    def ap(t, start, count, step, length):
        # channels [start + i*step : +length) for i in range(count), flat view
        if count == 1:
            return t[start : start + length, :]
        return bass.AP(
            t, start * F, [[step * F, count], [1, length * F]]
        )

    direct = [(y1, x1, g) for g in p1_groups] + [(y2, x0, g) for g in p2_groups]
    # round-robin across the two HWDGE rings, largest first for balance
    direct.sort(key=lambda d: -(d[2][1] * d[2][3]))
    direct_sp = direct[0::2]
    direct_act = direct[1::2]

    scr0 = scr1 = None
    if s_total:
        scr0 = nc.dram_tensor("scr0", [s_total, F], f32, kind="Internal")
        scr1 = nc.dram_tensor("scr1", [s_total, F], f32, kind="Internal")

    with (
        nc.semaphore("dma1") as s1,
        nc.semaphore("dma2") as s2,
        nc.Block() as block,
    ):

        @block.sync
        def _(sync):
            n = 0
            # stage the swap set first (reads of both buffers)
            o = 0
            for a, b in s_runs:
                k = b - a
                sync.dma_start(scr0[o : o + k, :], x0[a:b, :]).then_inc(s1, 16)
                sync.dma_start(scr1[o : o + k, :], x1[a:b, :]).then_inc(s1, 16)
                n += 32
                o += k
            n_stage = n
            # direct patches can go while staging drains
            for dst, src, (a, cnt, st, k) in direct_sp:
                sync.dma_start(
                    ap(dst, a, cnt, st, k), ap(src, a, cnt, st, k)
                ).then_inc(s1, 16)
                n += 16
            if s_total:
                # swap-set writes must wait for the staged reads
                sync.wait_ge(s1, n_stage)
                o = 0
                for a, b in s_runs:
                    k = b - a
                    sync.dma_start(y1[a:b, :], scr1[o : o + k, :]).then_inc(s1, 16)
                    sync.dma_start(y2[a:b, :], scr0[o : o + k, :]).then_inc(s1, 16)
                    n += 32
                    o += k
            if n:
                sync.wait_ge(s1, n)

        @block.scalar
        def _(scalar):
            n = 0
            for dst, src, (a, cnt, st, k) in direct_act:
                scalar.dma_start(
                    ap(dst, a, cnt, st, k), ap(src, a, cnt, st, k)
                ).then_inc(s2, 16)
                n += 16
            if n:
                scalar.wait_ge(s2, n)

    _strip_start_barrier(nc)
    return nc


def _strip_start_barrier(nc):
    """Drop the all-engine barrier bass emits between its preamble and user
    code.  Our DMAs depend on nothing from other engines (the const-AP
    memsets and register moves it orders are unused here), so SP/ACT can
    issue their patch DMAs ~1.5us earlier.  The barrier's gather/release
    semaphore ops are relative (dec/add), so removing the complete start
    wave leaves the end-of-program barrier balanced."""
    blk = nc.m.functions[0].blocks[0]
    assert blk.name == "main", blk.name
    kept = [
        i
        for i in blk.instructions
        if not (
            getattr(i, "name", "").startswith("barrier_")
            or type(i).__name__ == "InstDrain"
        )
    ]
    blk.instructions = kept


def _get_fn(key, m1, m2):
    cached = _FN_CACHE.get(key)
    if cached is not None:
        return cached

    import jax
    from jax.experimental.shard_map import shard_map
    from jax.sharding import Mesh, PartitionSpec as P

    from concourse.bass2jax import _bass_exec_p, install_neuronx_cc_hook

    install_neuronx_cc_hook()
    nc = _build_patch_program(m1, m2)
    aval = jax.core.ShapedArray((C, F), np.float32)

    def _body(a0, a1):
        outs = _bass_exec_p.bind(
            a0,
            a1,
            out_avals=(aval, aval),
            in_names=("x0", "x1"),
            out_names=("y1", "y2"),
            lowering_input_output_aliases=(),
            sim_require_finite=True,
            sim_require_nnan=True,
            nc=nc,
        )
        return tuple(outs)

    devices = jax.devices()[:N_CORES]
    assert len(devices) == N_CORES, f"need {N_CORES} cores, got {len(devices)}"
    mesh = Mesh(np.asarray(devices), ("core",))
    # donating x0/x1 makes jax alias them to the equal-shaped outputs
    # (y1<-x0, y2<-x1, first-fit in declaration order) — verified bit-exact.
    fn = jax.jit(
        shard_map(
            _body,
            mesh=mesh,
            in_specs=(P("core"), P("core")),
            out_specs=(P("core"), P("core")),
            check_rep=False,
        ),
        donate_argnums=(0, 1),
    )
    _FN_CACHE[key] = fn
    return fn


def kernel(x0, x1, w1, w2):
    x0 = np.ascontiguousarray(np.asarray(x0, dtype=np.float32))
    x1 = np.ascontiguousarray(np.asarray(x1, dtype=np.float32))
    assert x0.shape == (B, C, H, W) and x1.shape == (B, C, H, W)

    m1 = _mask(w1)
    m2 = _mask(w2)
    key = (m1.tobytes(), m2.tobytes())
    fn = _get_fn(key, m1, m2)
    o1, o2 = fn(x0.reshape(B * C, F), x1.reshape(B * C, F))
    y1 = np.asarray(o1).reshape(B, C, H, W)
    y2 = np.asarray(o2).reshape(B, C, H, W)
    return (y1, y2)
